# revision 1
# baseline (speedup 1.0000x reference)
"""Trainium2 Bass kernel for BayesLinearEMP (moe_routing).

out[b] = weights[mode_idx[b]] @ x[b] + biases[mode_idx[b]]
  x: [128, 2048] f32, weights: [20, 2048, 2048] f32, biases: [20, 2048] f32,
  mode_idx: [128] int

Strategy (8 NeuronCores):
  - Split the output dim O=2048 into 8 slices of 256, one per core.  Every
    core sees all 20 modes' weights for its O-slice: 42 MB/core, perfectly
    balanced regardless of the mode distribution (total weight traffic is
    read-once = 335 MB, the memory-roofline minimum).
  - On the host, sort samples by mode.  Per mode m with count c_m the core
    computes a [c_m, 256] tile as 16 K-chunk matmuls (K=128, N=256).
  - fp32 matmuls run at 1/4 PE rate, so each fp32 operand is split into a
    bf16 hi/lo pair ("pseudo-fp32"): W@x = Whi@xhi + Whi@xlo + Wlo@xhi
    (+ Wlo@xlo, dropped: ~2^-18 relative).  3 bf16 passes ≈ rel err ~1e-5
    at full PE rate; DMA traffic is unchanged vs fp32 (2x2B planes).
  - The bias is folded into the PSUM accumulation with a K=2 ones-matmul
    against the [bias_hi; bias_lo] planes.
"""

import sys

for _p in ("/opt/trn_rl_repo", "/root/.axon_site/_ro/trn_rl_repo"):
    if _p not in sys.path:
        sys.path.append(_p)

import numpy as np
import ml_dtypes

BF16 = ml_dtypes.bfloat16

B, I, O, M = 128, 2048, 2048, 20
NCORES = 8
OC = O // NCORES          # 256 output cols per core
KC = I // 128             # 16 contraction chunks

_CACHE: dict = {}
LAST_EXEC_TIME_NS = None


def _install_ntff_shim():
    """antenv.axon_hooks is absent in this image; recreate it so the
    trace=True path of run_bass_kernel_spmd can reach NTFF profiling."""
    import types
    import antenv

    if getattr(antenv, "axon_hooks", None) is not None:
        return
    hooks_mod = types.ModuleType("antenv.axon_hooks")
    _hook = [None]
    hooks_mod.set_axon_ntff_profile_hook = lambda h: _hook.__setitem__(0, h)
    hooks_mod.get_axon_ntff_profile_hook = lambda: _hook[0]
    sys.modules["antenv.axon_hooks"] = hooks_mod
    antenv.axon_hooks = hooks_mod
    try:
        from trn_agent_boot.trn_boot import _ntff_profile_via_ctypes

        hooks_mod.set_axon_ntff_profile_hook(
            _ntff_profile_via_ctypes("/opt/axon/libaxon_pjrt.so")
        )
    except Exception:
        pass
    import concourse.bass_utils as bass_utils

    bass_utils.upload_artifacts = lambda tmpdir: "local://" + tmpdir


def _build(counts: tuple) -> "bacc.Bacc":
    import concourse.bass as bass
    import concourse.tile as tile
    from concourse import bacc, mybir

    offs = np.concatenate([[0], np.cumsum(counts)]).astype(int)

    nc = bacc.Bacc("TRN2", target_bir_lowering=False, debug=False, num_devices=NCORES)
    bf = mybir.dt.bfloat16
    f32 = mybir.dt.float32

    wh_d = nc.dram_tensor("wh", [M, 128, 2 * KC * OC], bf, kind="ExternalInput").ap()
    xt_d = nc.dram_tensor("xt", [128, 2 * KC * 128], bf, kind="ExternalInput").ap()
    bh_d = nc.dram_tensor("bh", [2, M * OC], bf, kind="ExternalInput").ap()
    out_d = nc.dram_tensor("out", [B, OC], f32, kind="ExternalOutput").ap()

    with tile.TileContext(nc) as tc:
        with (
            tc.tile_pool(name="w", bufs=3) as wpool,
            tc.tile_pool(name="x", bufs=1) as xpool,
            tc.tile_pool(name="consts", bufs=1) as cpool,
            tc.tile_pool(name="o", bufs=3) as opool,
            tc.tile_pool(name="ps", bufs=4, space=bass.MemorySpace.PSUM) as pspool,
        ):
            xt = xpool.tile([128, 2 * KC * 128], bf)
            nc.sync.dma_start(xt[:], xt_d[:])
            bt = cpool.tile([2, M * OC], bf)
            nc.sync.dma_start(bt[:], bh_d[:])
            ones = cpool.tile([2, 128], bf)
            nc.vector.memset(ones[:], 1.0)

            # (wplane, xplane) terms of (Whi+Wlo) @ (xhi+xlo), Wlo@xlo dropped
            combos = ((0, 0), (0, 1), (1, 0))

            for m in range(M):
                cm = int(counts[m])
                if cm == 0:
                    continue
                o0 = int(offs[m])
                wt = wpool.tile([128, 2 * KC * OC], bf)
                nc.sync.dma_start(wt[:], wh_d[m])
                ps = pspool.tile([128, OC], f32)
                first = True
                for tw, tx in combos:
                    for k in range(KC):
                        xoff = (tx * KC + k) * 128 + o0
                        woff = (tw * KC + k) * OC
                        nc.tensor.matmul(
                            ps[0:cm, :],
                            xt[:, xoff : xoff + cm],
                            wt[:, woff : woff + OC],
                            start=first,
                            stop=False,
                        )
                        first = False
                nc.tensor.matmul(
                    ps[0:cm, :],
                    ones[:, 0:cm],
                    bt[:, m * OC : (m + 1) * OC],
                    start=False,
                    stop=True,
                )
                ot = opool.tile([128, OC], f32)
                nc.vector.tensor_copy(ot[0:cm, :], ps[0:cm, :])
                nc.scalar.dma_start(out_d[o0 : o0 + cm, :], ot[0:cm, :])

    nc.compile()
    return nc


def _hi_lo(a: np.ndarray):
    hi = a.astype(BF16)
    lo = (a - hi.astype(np.float32)).astype(BF16)
    return hi, lo


def kernel(x, weights, biases, mode_idx):
    global LAST_EXEC_TIME_NS
    import os

    x = np.asarray(x, dtype=np.float32)
    weights = np.asarray(weights, dtype=np.float32)
    biases = np.asarray(biases, dtype=np.float32)
    mode_idx_np = np.asarray(mode_idx).astype(np.int64)

    assert x.shape == (B, I) and weights.shape == (M, O, I)
    assert biases.shape == (M, O) and mode_idx_np.shape == (B,)

    order = np.argsort(mode_idx_np, kind="stable")
    counts = np.bincount(mode_idx_np, minlength=M)
    key = tuple(int(c) for c in counts)

    if key not in _CACHE:
        _CACHE[key] = _build(key)
    nc = _CACHE[key]

    # --- host-side data prep into the on-chip layouts ---
    xs = x[order]                                    # [B, I] sorted by mode
    xhi, xlo = _hi_lo(xs)
    xpl = np.stack([xhi, xlo], 0)                    # [t, s, i]
    XT = np.ascontiguousarray(
        xpl.reshape(2, B, KC, 128).transpose(3, 0, 2, 1)   # [p, t, k, s]
    ).reshape(128, 2 * KC * 128)

    whi, wlo = _hi_lo(weights)
    wpl = np.stack([whi, wlo], 0)                    # [t, m, o, i]
    WH = np.ascontiguousarray(
        wpl.reshape(2, M, NCORES, OC, KC, 128).transpose(2, 1, 5, 0, 4, 3)
    ).reshape(NCORES, M, 128, 2 * KC * OC)           # [c, m, p, (t,k,cc)]

    bhi, blo = _hi_lo(biases)
    bpl = np.stack([bhi, blo], 0)                    # [t, m, o]
    BH = np.ascontiguousarray(
        bpl.reshape(2, M, NCORES, OC).transpose(2, 0, 1, 3)
    ).reshape(NCORES, 2, M * OC)

    in_maps = [{"wh": WH[c], "xt": XT, "bh": BH[c]} for c in range(NCORES)]

    from concourse.bass_utils import run_bass_kernel_spmd

    trace = bool(int(os.environ.get("BASS_KERNEL_TRACE", "0")))
    if trace:
        _install_ntff_shim()
    res = run_bass_kernel_spmd(
        nc,
        in_maps,
        list(range(NCORES)),
        trace=trace,
        trace_cores=list(range(NCORES)) if trace else None,
    )
    LAST_EXEC_TIME_NS = res.exec_time_ns

    sorted_out = np.concatenate(
        [res.results[c]["out"] for c in range(NCORES)], axis=1
    )                                                # [B, O] in sorted order
    out = np.empty((B, O), dtype=np.float32)
    out[order] = sorted_out
    return out
